# revision 40
# baseline (speedup 1.0000x reference)
"""MoE gate (DeepSeek-style) Bass/Tile kernel for 8 Trainium2 NeuronCores.

Problem: hidden_states [4, 4096, 2048] fp32, gate weight [64, 2048] fp32.
  logits = x @ W.T            -> [16384, 64]
  scores = softmax(logits)    -> top-8 (values renormalized) + indices
  aux    = seq-aux loss scalar

Sharding: data-parallel over tokens. Core c gets tokens
[c*2048, (c+1)*2048) -- exactly half of one batch's sequence, so each
core's aux-loss partial sums (top-8 counts per expert, score sums per
expert) belong to a single batch. The [2,64] partials are combined on the
host into the scalar aux loss (a ~1e3-flop finish).

Device pipeline per 128-token tile:
  DMA x [128,2048] (natural, contiguous) -> PE transpose-mode (fp32,
  bit-exact) produces xT chunks in PSUM -> DVE/ACT copy to SBUF ->
  16 accumulated fp32 matmuls vs pre-transposed W.T -> logits [128,64]
  (tokens on partitions) -> DVE max8/max_index/match_replace (exact
  jax-style descending top-8 with lowest-index tie-break) -> ACT exp with
  fused row-sum -> DVE normalize; per-partition accumulators for the
  aux partials, reduced across partitions at the end with a ones-matmul.
"""

from contextlib import ExitStack

import numpy as np

import concourse.bass as bass
import concourse.mybir as mybir
import concourse.tile as tile
from concourse.masks import make_identity

# ---------------------------------------------------------------------------
# Workaround for the walrus pin in this container: it rejects instructions
# carrying more than one sync-wait command ("Too many sync wait commands").
# Tile attaches multi-waits both to regular instructions (stage 1B) and to
# the kernel-tail drain. Split them onto single-wait carrier nops on the
# same engine, placed immediately before the instruction; tile semaphores
# are monotonic within the kernel body so sequential waits are equivalent.
# ---------------------------------------------------------------------------
from bass_rust import ScopedClock, SyncInfo

_orig_lower = tile.TileContext._lower_ordered_insts


def _split_waits_in_ordered(ordered):
    for bb_name, insts in list(ordered.items()):
        new_insts = []
        for inst in insts:
            try:
                si = inst.sync_info
            except AttributeError:
                si = None
            if si is not None and len(si.on_wait) > 1:
                waits = list(si.on_wait)
                updates = list(si.on_update)
                for k, w in enumerate(waits[:-1]):
                    new_insts.append(
                        mybir.InstNoOp(
                            name=f"{inst.name}-wsplit{k}",
                            engine=inst.engine,
                            ins=[],
                            outs=[],
                            sync_info=SyncInfo(on_wait=[w], on_update=[]),
                        )
                    )
                inst.sync_info = SyncInfo(on_wait=[waits[-1]], on_update=updates)
            new_insts.append(inst)
        ordered[bb_name] = new_insts


def _patched_lower(self, ordered):
    _split_waits_in_ordered(ordered)
    return _orig_lower(self, ordered)


def _patched_drain_and_barrier(self, tick_clock, wait_clock):
    nc = self.nc
    drain_inst = nc.sync.drain()
    wait_clock.add_sem_waits(
        drain_inst.ins, ScopedClock({None: tick_clock.global_clock})
    )
    si = drain_inst.ins.sync_info
    if si is not None and len(si.on_wait) > 1:
        waits = list(si.on_wait)
        updates = list(si.on_update)
        drain_inst.ins.sync_info = SyncInfo(on_wait=[waits[0]], on_update=updates)
        for w in waits[1:]:
            carrier = nc.sync.nop(nofuse=True, hint="drain_wait_split")
            carrier.ins.sync_info = SyncInfo(on_wait=[w], on_update=[])

    nc.all_engine_barrier()
    assert self.sems is not None
    popped = nc._tile_sem_poison_stack.pop()
    assert popped is self._sem_poison
    nc.clear_and_free_semaphores(list(self.sems.allocated().values()))
    nc.all_engine_barrier()


tile.TileContext._drain_and_barrier = _patched_drain_and_barrier
tile.TileContext._lower_ordered_insts = _patched_lower

# ---------------------------------------------------------------------------

N_CORES = 8
B, S, D, E = 4, 4096, 2048, 64
TOP_K = 8
ALPHA = 1e-3
T_TOTAL = B * S
T_CORE = T_TOTAL // N_CORES  # 2048 tokens per core
P = 128                      # tokens per tile (partition dim)
N_TILES = T_CORE // P        # 16
N_CHUNK = D // P             # 16 contraction chunks
F32 = mybir.dt.float32
U32 = mybir.dt.uint32

_cache = {}

# tuning knobs (overridable before _build)
CFG = {
    "x_bufs": 3,
    "xt_bufs": 2,
    "pt_bufs": 4,
    "pl_bufs": 4,
    "copy_eng": "vaav",  # per-quarter engine: v=DVE a=ACT
    "lg_eng": "a",
    "x0_act": False,
    "copy_hiprio": 0,
    "bank_cols": 512,
    "store_two_queues": False,
    "small_bufs": 3,
    "split_mm": False,
    "out8_bufs": 3,
    "part_queue": "sync",
    "pe_warmup": 32,
    "warmup_noident": False,
    "w_chunks": 2,
    "mask_gpsimd": False,
    "x0_first": False,
    "x_chunks": 4,
    "split_stores": True,
}


def _build(n_reps: int = 1):
    nc = bass.Bass("TRN2", target_bir_lowering=False, debug=False, num_devices=1)

    x_d = nc.dram_tensor("x", [T_CORE, D], F32, kind="ExternalInput")
    w_d = nc.dram_tensor("w", [E, D], F32, kind="ExternalInput")
    idx_d = nc.dram_tensor("idx", [T_CORE, TOP_K], U32, kind="ExternalOutput")
    wt_d = nc.dram_tensor("wt", [T_CORE, TOP_K], F32, kind="ExternalOutput")
    part_d = nc.dram_tensor("part", [2, E], F32, kind="ExternalOutput")

    with tile.TileContext(nc) as tc, ExitStack() as ctx:
        consts = ctx.enter_context(tc.tile_pool(name="consts", bufs=1))
        xpool = ctx.enter_context(tc.tile_pool(name="x", bufs=CFG["x_bufs"]))
        xtpool = ctx.enter_context(tc.tile_pool(name="xt", bufs=CFG["xt_bufs"]))
        small = ctx.enter_context(tc.tile_pool(name="small", bufs=CFG["small_bufs"]))
        out8 = ctx.enter_context(tc.tile_pool(name="out8", bufs=CFG["out8_bufs"]))
        pt = ctx.enter_context(tc.tile_pool(name="pt", bufs=CFG["pt_bufs"], space="PSUM"))
        pl = ctx.enter_context(tc.tile_pool(name="pl", bufs=CFG["pl_bufs"], space="PSUM"))

        ident = consts.tile([P, P], F32)
        make_identity(nc, ident[:])
        ones = consts.tile([P, 1], F32)
        nc.vector.memset(ones[:], 1.0)

        if isinstance(n_reps, tuple):  # ("loop", n) -> hardware loop timing variant
            n = n_reps[1]
            with tc.For_i(0, n, 1):
                _body(nc, tc, consts, xpool, xtpool, small, out8, pt, pl,
                      ident, ones, x_d, w_d, idx_d, wt_d, part_d)
        else:
            for _rep in range(n_reps):
                _body(nc, tc, consts, xpool, xtpool, small, out8, pt, pl,
                      ident, ones, x_d, w_d, idx_d, wt_d, part_d)

    return nc


def _body(nc, tc, consts, xpool, xtpool, small, out8, pt, pl,
          ident, ones, x_d, w_d, idx_d, wt_d, part_d):
    if True:
        # ---- PE warmup: tiny matmuls to lift the HAM clock gate while the
        # first DMAs are in flight ----
        if CFG["pe_warmup"]:
            wu = pt.tile([P, 512], F32, tag="xt")
            if CFG["warmup_noident"]:
                wsrc = consts.tile([P, 16], F32)
                nc.vector.memset(wsrc[:], 0.0)  # cheap DVE producer, no gpsimd dep
            else:
                wsrc = ident
            for i in range(CFG["pe_warmup"]):
                out_ap = (wu[0:16, (i % 16) * 32:(i % 16) * 32 + 16]
                          if CFG["warmup_noident"]
                          else wu[:, (i % 16) * 32:(i % 16) * 32 + 16])
                nc.tensor.matmul(
                    out_ap,
                    lhsT=wsrc[:, 0:16] if CFG["warmup_noident"] else ident[:],
                    rhs=wsrc[:, 0:16],
                    start=True, stop=True,
                )

        # ---- one-time: W [64, 2048] -> W.T chunks wt_sb[:, 64c:64c+64] ----
        x_nat0 = None
        if CFG["x0_first"]:
            x_nat0 = xpool.tile([P, D], F32, tag="x")
            nc.sync.dma_start(x_nat0[:], x_d.ap()[0:P, :])

        w_nat = consts.tile([E, D], F32)
        if CFG["w_chunks"] > 1:
            wc = D // CFG["w_chunks"]
            for i in range(CFG["w_chunks"]):
                nc.sync.dma_start(w_nat[:, i * wc:(i + 1) * wc],
                                  w_d.ap()[:, i * wc:(i + 1) * wc])
        else:
            nc.sync.dma_start(w_nat[:], w_d.ap())
        wt_sb = consts.tile([P, N_CHUNK * E], F32)
        for h in range(2):
            ptile = pt.tile([P, 512], F32, tag="xt")
            for j in range(8):
                c = h * 8 + j
                nc.tensor.transpose(
                    ptile[:, j * E:(j + 1) * E],
                    w_nat[:, c * P:(c + 1) * P],
                    ident[:E, :E],
                )
            if h == 0:
                nc.vector.tensor_copy(wt_sb[:, h * 512:(h + 1) * 512], ptile[:])
            else:
                nc.scalar.copy(wt_sb[:, h * 512:(h + 1) * 512], ptile[:])

        # ---- per-partition accumulators for aux partials ----
        acc_cnt = consts.tile([P, E], F32)
        nc.vector.memset(acc_cnt[:], 0.0)
        acc_sc = consts.tile([P, E], F32)
        nc.vector.memset(acc_sc[:], 0.0)
        i8_all = consts.tile([P, N_TILES * TOP_K], U32)
        w8_all = consts.tile([P, N_TILES * TOP_K], F32)

        for t in range(N_TILES):
            if t == 0 and x_nat0 is not None:
                x_nat = x_nat0
            else:
                x_nat = xpool.tile([P, D], F32, tag="x")
                xq = nc.scalar if (t == 0 and CFG["x0_act"]) else nc.sync
                nxc = CFG["x_chunks"]
                if nxc > 1:
                    xc = D // nxc
                    for i in range(nxc):
                        xq.dma_start(x_nat[:, i * xc:(i + 1) * xc],
                                     x_d.ap()[t * P:(t + 1) * P, i * xc:(i + 1) * xc])
                else:
                    xq.dma_start(x_nat[:], x_d.ap()[t * P:(t + 1) * P, :])

            # transpose x tile: 16 chunks of [128,128] -> 4 PSUM banks of
            # [128,512]; copy each bank to SBUF (alternating DVE/ACT)
            xt_sb = xtpool.tile([P, D], F32)
            bc = CFG["bank_cols"]
            nq = D // bc
            trs_per = bc // P
            for q in range(nq):
                ptile = pt.tile([P, bc], F32, tag="xt")
                for j in range(trs_per):
                    c = q * trs_per + j
                    nc.tensor.transpose(
                        ptile[:, j * P:(j + 1) * P],
                        x_nat[:, c * P:(c + 1) * P],
                        ident[:],
                    )
                eng = CFG["copy_eng"][q % len(CFG["copy_eng"])]
                if eng == "v":
                    nc.vector.tensor_copy(xt_sb[:, q * bc:(q + 1) * bc], ptile[:])
                else:
                    nc.scalar.copy(xt_sb[:, q * bc:(q + 1) * bc], ptile[:])

            # logits [128 tokens, 64 experts], fp32, accumulated over chunks
            if CFG["split_mm"]:
                lgA = pl.tile([P, E], F32, tag="lg")
                lgB = pl.tile([P, E], F32, tag="lgB")
                H = N_CHUNK // 2
                for c in range(N_CHUNK):
                    dst = lgA if c < H else lgB
                    nc.tensor.matmul(
                        dst[:],
                        lhsT=xt_sb[:, c * P:(c + 1) * P],
                        rhs=wt_sb[:, c * E:(c + 1) * E],
                        start=(c % H == 0),
                        stop=(c % H == H - 1),
                    )
                lg_ps = lgA
                nc.vector.tensor_add(lg_ps[:], lgA[:], lgB[:])
            else:
                lg_ps = pl.tile([P, E], F32, tag="lg")
                for c in range(N_CHUNK):
                    nc.tensor.matmul(
                        lg_ps[:],
                        lhsT=xt_sb[:, c * P:(c + 1) * P],
                        rhs=wt_sb[:, c * E:(c + 1) * E],
                        start=(c == 0),
                        stop=(c == N_CHUNK - 1),
                    )
            lg = small.tile([P, E], F32, tag="lg_sb")
            if CFG["lg_eng"] == "a":
                nc.scalar.copy(lg[:], lg_ps[:])
            else:
                nc.vector.tensor_copy(lg[:], lg_ps[:])

            # top-8 (descending, lowest-index tie-break = jax.lax.top_k)
            m8 = out8.tile([P, 8], F32, tag="m8")
            nc.vector.max(m8[:], lg[:])
            i8 = i8_all[:, t * TOP_K:(t + 1) * TOP_K]
            nc.vector.max_index(i8, m8[:], lg[:])

            # exact top-8 membership mask via match_replace (counts for aux)
            repl = small.tile([P, E], F32, tag="repl")
            nc.vector.match_replace(
                repl[:], in_to_replace=m8[:], in_values=lg[:], imm_value=-1e30
            )
            mask = small.tile([P, E], F32, tag="mask")
            meng = nc.gpsimd if CFG["mask_gpsimd"] else nc.vector
            meng.tensor_tensor(mask[:], lg[:], repl[:], mybir.AluOpType.not_equal)
            meng.tensor_add(acc_cnt[:], acc_cnt[:], mask[:])

            # softmax pieces (shift by row max = m8[:,0])
            negmax = out8.tile([P, 1], F32, tag="negmax")
            nc.vector.tensor_scalar_mul(negmax[:], m8[:, 0:1], -1.0)
            exp8 = out8.tile([P, 8], F32, tag="exp8")
            sum8 = out8.tile([P, 1], F32, tag="sum8")
            nc.scalar.activation(
                exp8[:], m8[:], mybir.ActivationFunctionType.Exp,
                bias=negmax[:], scale=1.0, accum_out=sum8[:],
            )
            expf = small.tile([P, E], F32, tag="expf")
            sumf = out8.tile([P, 1], F32, tag="sumf")
            nc.scalar.activation(
                expf[:], lg[:], mybir.ActivationFunctionType.Exp,
                bias=negmax[:], scale=1.0, accum_out=sumf[:],
            )

            r8 = out8.tile([P, 1], F32, tag="r8")
            nc.vector.reciprocal(r8[:], sum8[:])
            rf = out8.tile([P, 1], F32, tag="rf")
            nc.vector.reciprocal(rf[:], sumf[:])

            w8 = w8_all[:, t * TOP_K:(t + 1) * TOP_K]
            nc.vector.tensor_scalar_mul(w8, exp8[:], r8[:, 0:1])

            # acc_sc += expf * rf  (fused multiply-add on DVE)
            nc.vector.scalar_tensor_tensor(
                out=acc_sc[:], in0=expf[:], scalar=rf[:, 0:1], in1=acc_sc[:],
                op0=mybir.AluOpType.mult, op1=mybir.AluOpType.add,
            )

        # ---- batched output stores: [128, t, 8] -> DRAM [(t 128), 8] ----
        if CFG["split_stores"]:
            H = N_TILES // 2
            wq = nc.scalar if CFG["store_two_queues"] else nc.sync
            for h in range(2):
                io = idx_d.ap()[h * H * P:(h + 1) * H * P, :]
                wo = wt_d.ap()[h * H * P:(h + 1) * H * P, :]
                nc.sync.dma_start(io.rearrange("(t p) k -> p t k", t=H),
                                  i8_all[:, h * H * TOP_K:(h + 1) * H * TOP_K])
                wq.dma_start(wo.rearrange("(t p) k -> p t k", t=H),
                             w8_all[:, h * H * TOP_K:(h + 1) * H * TOP_K])
        else:
            nc.sync.dma_start(
                idx_d.ap().rearrange("(t p) k -> p t k", t=N_TILES), i8_all[:]
            )
            nc.sync.dma_start(
                wt_d.ap().rearrange("(t p) k -> p t k", t=N_TILES), w8_all[:]
            )

        # ---- cross-partition reduction of aux partials ----
        red = pl.tile([1, E], F32, tag="lg")
        part_sb = small.tile([1, 2 * E], F32, tag="part")
        nc.tensor.matmul(red[:], lhsT=ones[:], rhs=acc_cnt[:], start=True, stop=True)
        nc.vector.tensor_copy(part_sb[:, 0:E], red[:])
        red2 = pl.tile([1, E], F32, tag="lg")
        nc.tensor.matmul(red2[:], lhsT=ones[:], rhs=acc_sc[:], start=True, stop=True)
        nc.vector.tensor_copy(part_sb[:, E:2 * E], red2[:])
        pq = {"act": nc.scalar, "pool": nc.gpsimd, "sync": nc.sync}[CFG["part_queue"]]
        pq.dma_start(part_d.ap(), part_sb[:])


def _get_nc():
    if "nc" not in _cache:
        _cache["nc"] = _build()
    return _cache["nc"]


def kernel(hidden_states: np.ndarray, weight: np.ndarray):
    from concourse.bass_utils import run_bass_kernel_spmd

    x = np.ascontiguousarray(hidden_states.reshape(T_TOTAL, D), dtype=np.float32)
    w = np.ascontiguousarray(weight, dtype=np.float32)

    nc = _get_nc()
    in_maps = [
        {"x": x[c * T_CORE:(c + 1) * T_CORE], "w": w} for c in range(N_CORES)
    ]
    res = run_bass_kernel_spmd(nc, in_maps, core_ids=list(range(N_CORES)))

    topk_idx = np.concatenate(
        [res.results[c]["idx"].astype(np.int32) for c in range(N_CORES)], axis=0
    )
    topk_weight = np.concatenate(
        [res.results[c]["wt"] for c in range(N_CORES)], axis=0
    )

    # host finish for the aux-loss scalar: combine per-core [2, E] partials
    parts = np.stack([res.results[c]["part"] for c in range(N_CORES)])  # [8,2,E]
    aux = 0.0
    for b in range(B):
        cnt = parts[2 * b, 0] + parts[2 * b + 1, 0]          # [E]
        ssum = parts[2 * b, 1] + parts[2 * b + 1, 1]         # [E]
        ce = cnt / (S * TOP_K / E)
        smean = ssum / S
        aux += float((ce * smean).sum())
    aux_loss = np.float32(aux / B * ALPHA)

    return topk_idx, topk_weight, aux_loss
